# revision 76
# baseline (speedup 1.0000x reference)
"""Trainium2 Bass kernel for multi-head attention (B=4, H=8, L=2048, dim=512).

Sharding: 8 cores = 4 batches x 2 query halves. Each core computes attention
for one batch's 1024-query half (all 8 heads) over the full 2048-key range;
no cross-core communication.

Design: the wall clock is the ScalarE exp stream (16.8M softmax elements at
1 elem/cycle/lane); everything else hides under it:
  - Q/K/V projections interleave into the attention kt-loop as PE filler;
    a short warm-up matmul burst releases the PE HAM clock-gate before the
    prologue projections run.
  - Scores: row-packed pairs (two 64-contraction matmuls in row groups 0-1 /
    2-3); attn@V: col-packed pairs accumulating PSUM over kt. PE order is
    S(kt+1) before AV(kt) with a 3-slot score-PSUM ring so the PE refills
    while exp(kt) drains.
  - exp tiles land in paired [128, 2, 1024] fp16 SBUF tiles so the L1
    denominator adds process both head-halves in one DVE op.
  - Denominator: L1 pairwise adds (8 per m) -> S1[8 slots]; the remaining
    reduction is an accumulating all-ones [128,64] stationary matmul over
    the 8 slots (PE), whose output rows all equal the key-sum, followed by
    fast reciprocal + normalize on DVE. Deferred into the next m's kt 2..3.
  - A tunable subset of (m,kt) A-halves uses a Schraudolph bit-trick exp
    (round(1477.32*s + 15300) as int16 == fp16 bits): DVE casts PSUM->fp16,
    GpSimd does the fused mul+add+round. ~4% max elementwise error on those
    tiles, <1e-2 on the final output.
"""
import numpy as np

import concourse.bass as bass
import concourse.tile as tile
from concourse import bacc, mybir
from concourse.bass_utils import run_bass_kernel_spmd

F16 = mybir.dt.float16
F32 = mybir.dt.float32
I16 = mybir.dt.int16
EXP = mybir.ActivationFunctionType.Exp
CPY = mybir.ActivationFunctionType.Copy
MUL = mybir.AluOpType.mult
ADD = mybir.AluOpType.add

P = 128
D = 512          # model dim
L = 2048         # keys
QL = 1024        # per-core queries
H = 8
C = 64           # head dim
HID = 512
DC = D // P      # 4 contraction chunks
KT = L // P      # 16 key tiles
N = 512          # matmul free-dim chunk
QC = QL // N     # 2 query chunks
LC = L // N      # 4 key chunks
SCALE = C ** -0.5

A_SCH = 1477.319722        # 1024/ln(2)
B_SCH = 15300.0            # fp16 exp bias 15360 - sigma* (60)

# kt's whose A-half exp uses the approximate bit-trick path, computed
# entirely on DVE straight from PSUM; B-half stays exact on ScalarE.
APPROX_KT = frozenset({4, 5, 6, 8, 10, 11, 14})


def emit(nc, tc, x, wq, wk, wv, wo, bias, out):
    import contextlib
    ctx = contextlib.ExitStack()
    with ctx:
        # ---- pools -----------------------------------------------------
        consts = ctx.enter_context(tc.tile_pool(name="consts", bufs=1))
        qkv = ctx.enter_context(tc.tile_pool(name="qkv", bufs=1))
        ph1 = ctx.enter_context(tc.tile_pool(name="ph1", bufs=1))
        atp = ctx.enter_context(tc.tile_pool(name="atp", bufs=4))
        stp = ctx.enter_context(tc.tile_pool(name="stp", bufs=2))
        s1p = ctx.enter_context(tc.tile_pool(name="s1p", bufs=1))
        s2p = ctx.enter_context(tc.tile_pool(name="s2p", bufs=2))
        s3p = ctx.enter_context(tc.tile_pool(name="s3p", bufs=2))
        s4p = ctx.enter_context(tc.tile_pool(name="s4p", bufs=2))
        rbp = ctx.enter_context(tc.tile_pool(name="rbp", bufs=2))
        otup = ctx.enter_context(tc.tile_pool(name="otup", bufs=2))
        outp = ctx.enter_context(tc.tile_pool(name="outp", bufs=2))
        # PSUM: shared 3-slot ring (6 banks) for scores + projection/rb/
        # out-proj tiles, + po 2 banks = 8 banks.
        pps = ctx.enter_context(tc.tile_pool(name="pps", bufs=3, space="PSUM"))
        ppo = ctx.enter_context(tc.tile_pool(name="ppo", bufs=1, space="PSUM"))

        # ---- persistent SBUF ------------------------------------------
        wo_sb = consts.tile([P, DC, HID], F16)
        bias_sb = consts.tile([P, DC], F32)
        ones_sb = consts.tile([P, C], F16)
        warm_sb = consts.tile([P, N], F16)
        nc.vector.memset(ones_sb[:], 1.0)
        nc.vector.memset(warm_sb[:], 0.25)
        # load the exp table while DMAs stream
        nc.scalar.activation(warm_sb[:, 0:1], warm_sb[:, 0:1], EXP)

        q_sb = qkv.tile([P, DC, QL], F16)
        k_sb = qkv.tile([P, DC, L], F16)
        vt_sb = qkv.tile([P, KT, HID], F16)
        ot_sb = qkv.tile([P, DC, QL], F16)

        x_sb = ph1.tile([P, DC, L], F16)
        wq_sb = ph1.tile([P, DC, HID], F16)
        wk_sb = ph1.tile([P, DC, HID], F16)
        wv_sb = ph1.tile([P, DC, HID], F16)
        xr = x.rearrange("(a p) n -> p a n", p=P)
        nc.sync.dma_start(out=wq_sb[:], in_=wq.rearrange("(a p) n -> p a n", p=P))
        nc.sync.dma_start(out=x_sb[:, :, 0:N], in_=xr[:, :, 0:N])
        nc.sync.dma_start(out=x_sb[:, :, N:QL], in_=xr[:, :, N:QL])
        nc.sync.dma_start(out=wk_sb[:], in_=wk.rearrange("(a p) n -> p a n", p=P))
        nc.sync.dma_start(out=wv_sb[:], in_=wv.rearrange("(a p) n -> p a n", p=P))
        nc.sync.dma_start(out=x_sb[:, :, QL:QL + N], in_=xr[:, :, QL:QL + N])
        nc.sync.dma_start(out=x_sb[:, :, QL + N:L], in_=xr[:, :, QL + N:L])
        nc.sync.dma_start(out=wo_sb[:], in_=wo.rearrange("(a p) n -> p a n", p=P))
        nc.sync.dma_start(out=bias_sb[:], in_=bias)

        # HAM warm-up: ~9 dummy matmuls (~3.5us cold) release the PE clock
        # gate so the prologue projections run at 2.4 GHz. Results unused.
        warm_ps = pps.tile([P, N], F32, tag="ps", name="warm_ps")
        for i in range(9):
            nc.tensor.matmul(
                warm_ps[:], lhsT=warm_sb[:, 0:P], rhs=warm_sb[:],
                start=(i == 0), stop=(i == 8),
            )

        # ---- projection groups (PE filler) -----------------------------
        def q_proj(m, qc):
            ps = pps.tile([P, N], F32, tag="ps", name=f"q{m}{qc}")
            for dc in range(DC):
                nc.tensor.matmul(
                    ps[:], lhsT=wq_sb[:, dc, m * P:(m + 1) * P],
                    rhs=x_sb[:, dc, qc * N:(qc + 1) * N],
                    start=(dc == 0), stop=(dc == DC - 1),
                )
            nc.vector.tensor_copy(q_sb[:, m, qc * N:(qc + 1) * N], ps[:])

        def k_proj(m, lc):
            ps = pps.tile([P, N], F32, tag="ps", name=f"k{m}{lc}")
            for dc in range(DC):
                nc.tensor.matmul(
                    ps[:], lhsT=wk_sb[:, dc, m * P:(m + 1) * P],
                    rhs=x_sb[:, dc, lc * N:(lc + 1) * N],
                    start=(dc == 0), stop=(dc == DC - 1),
                )
            nc.vector.tensor_copy(k_sb[:, m, lc * N:(lc + 1) * N], ps[:])

        def vt_proj(kt):
            ps = pps.tile([P, N], F32, tag="ps", name=f"vt{kt}")
            for dc in range(DC):
                nc.tensor.matmul(
                    ps[:], lhsT=x_sb[:, dc, kt * P:(kt + 1) * P],
                    rhs=wv_sb[:, dc, :],
                    start=(dc == 0), stop=(dc == DC - 1),
                )
            # DVE-only: m0's ScalarE is saturated by the exp stream; casts
            # there would make it the m0 bottleneck
            nc.vector.tensor_copy(vt_sb[:, kt, :], ps[:])

        fill = {}

        def add_fill(m, kt, fn, *a):
            fill.setdefault((m, kt), []).append((fn, a))

        # fillers sit on approx kts where ScalarE idles and score PSUM
        # frees fast; K(m,lc) must land before kt=4*lc.
        add_fill(0, 1, k_proj, 0, 1)
        add_fill(0, 5, k_proj, 0, 2)
        add_fill(0, 8, k_proj, 0, 3)
        add_fill(0, 11, q_proj, 1, 0)
        add_fill(0, 13, q_proj, 1, 1)
        add_fill(0, 14, k_proj, 1, 0)
        for m in (1, 2):
            add_fill(m, 1, k_proj, m, 1)
            add_fill(m, 5, k_proj, m, 2)
            add_fill(m, 8, k_proj, m, 3)
            add_fill(m, 11, q_proj, m + 1, 0)
            add_fill(m, 14, q_proj, m + 1, 1)
            add_fill(m, 6, k_proj, m + 1, 0)
        add_fill(3, 1, k_proj, 3, 1)
        add_fill(3, 5, k_proj, 3, 2)
        add_fill(3, 8, k_proj, 3, 3)

        # ---- exp dispatch ----------------------------------------------
        def emit_exp(m, kt, ps_t, at_t, half):
            dst = at_t[:, half, :]
            if half == 0 and kt in APPROX_KT:
                nc.vector.tensor_scalar(
                    dst.bitcast(I16), ps_t[:], A_SCH, B_SCH, MUL, ADD,
                )
            else:
                nc.scalar.activation(dst, ps_t[:], EXP)

        def scores_half(m, kt, half):
            # one row-packed half (A: rows 0:64 at (0,0), B: 64:128 at (64,0))
            lo = half * C
            ps_t = pps.tile([P, QL], F32, tag="ps", name=f"ps{half}")
            for qc in range(QC):
                nc.tensor.matmul(
                    ps_t[:, qc * N:(qc + 1) * N],
                    lhsT=k_sb[lo:lo + C, m, kt * P:(kt + 1) * P],
                    rhs=q_sb[lo:lo + C, m, qc * N:(qc + 1) * N],
                    start=True, stop=True, tile_position=(lo, 0),
                )
            return ps_t

        def scores_pair(m, kt):
            # interleave A/B so the row-packed pairs stream concurrently
            # (used on kts whose ring slots are double-buffered)
            ps_A = pps.tile([P, QL], F32, tag="ps")
            ps_B = pps.tile([P, QL], F32, tag="ps")
            for qc in range(QC):
                nc.tensor.matmul(
                    ps_A[:, qc * N:(qc + 1) * N],
                    lhsT=k_sb[0:C, m, kt * P:(kt + 1) * P],
                    rhs=q_sb[0:C, m, qc * N:(qc + 1) * N],
                    start=True, stop=True, tile_position=(0, 0),
                )
                nc.tensor.matmul(
                    ps_B[:, qc * N:(qc + 1) * N],
                    lhsT=k_sb[C:P, m, kt * P:(kt + 1) * P],
                    rhs=q_sb[C:P, m, qc * N:(qc + 1) * N],
                    start=True, stop=True, tile_position=(C, 0),
                )
            return ps_A, ps_B

        def attnv(m, kt, po, at_t):
            for qc in range(QC):
                nc.tensor.matmul(
                    po[0:C, qc * N:(qc + 1) * N],
                    lhsT=vt_sb[:, kt, (2 * m) * C:(2 * m + 1) * C],
                    rhs=at_t[:, 0, qc * N:(qc + 1) * N],
                    start=(kt == 0), stop=(kt == KT - 1),
                    tile_position=(0, 0), skip_group_check=True,
                )
                nc.tensor.matmul(
                    po[C:P, qc * N:(qc + 1) * N],
                    lhsT=vt_sb[:, kt, (2 * m + 1) * C:(2 * m + 2) * C],
                    rhs=at_t[:, 1, qc * N:(qc + 1) * N],
                    start=(kt == 0), stop=(kt == KT - 1),
                    tile_position=(0, C), skip_group_check=True,
                )

        def finish_tail(st, kt_slot, last=False):
            # deferred denominators: accumulate the fold outputs through the
            # all-ones stationary matmul (rows land pre-broadcast), then
            # fast reciprocal + normalize. The non-deferred last-m tail
            # reads s2/s1 directly so the final folds stay off its chain.
            m, otu, s2, s1, s3, rbr = st
            qc = kt_slot - 2
            if last:
                srcs = [(s2, 0), (s2, 1), (s2, 2), (s1, 6), (s1, 7)]
            else:
                srcs = [(s3, 0)]
            rb_ps = pps.tile([P, N], F32, tag="ps", name=f"rb{m}_{qc}")
            for step, (src, j) in enumerate(srcs):
                for half in range(2):
                    nc.tensor.matmul(
                        rb_ps[half * C:(half + 1) * C, :],
                        lhsT=ones_sb[:],
                        rhs=src[:, j, half, qc * N:(qc + 1) * N],
                        start=(step == 0), stop=(step == len(srcs) - 1),
                        tile_position=(0, half * C), skip_group_check=True,
                    )
            nc.vector.reciprocal_approx_fast(
                out=rbr[:, qc * N:(qc + 1) * N], in_=rb_ps[:]
            )
            nc.vector.tensor_mul(
                ot_sb[:, m, qc * N:(qc + 1) * N], otu[:, qc * N:(qc + 1) * N],
                rbr[:, qc * N:(qc + 1) * N],
            )

        # ---- prologue (vt(1) rides inside the kt loop, under the first
        # exps, instead of extending the exp-idle prologue) ----------------
        q_proj(0, 0)
        q_proj(0, 1)
        k_proj(0, 0)
        vt_proj(0)

        # ---- main loop --------------------------------------------------
        pending = None
        at_next = None
        for m in range(DC):
            po = ppo.tile([P, QL], F32, tag="po", name=f"po{m}")
            s1 = s1p.tile([P, KT // 2, 2, QL], F16, tag="s1", name=f"s1_{m}")
            s2 = s2p.tile([P, KT // 4, 2, QL], F16, tag="s2", name=f"s2_{m}")
            at_prev = None
            if at_next is None:
                ps_A0, ps_B0 = scores_pair(m, 0)
                at_next = atp.tile([P, 2, QL], F16, tag="at")
                emit_exp(m, 0, ps_A0, at_next, 0)
                emit_exp(m, 0, ps_B0, at_next, 1)
            for kt in range(KT):
                at_t = at_next
                # refill score PSUM for kt+1 (or next m's kt 0) before AV(kt).
                # On filler kts, fillers are emitted BETWEEN the A-half and
                # B-half of the next scores: the PE FIFO stalls there anyway
                # (B waits exp_B of the previous kt), so the filler matmuls
                # execute inside that window instead of queueing behind it.
                # On clean kts the interleaved pair keeps row-pack concurrency.
                nxt = None
                if kt < KT - 1:
                    nxt = (m, kt + 1)
                elif m < DC - 1:
                    nxt = (m + 1, 0)
                fills = list(fill.get((m, kt), ()))
                has_vt = m == 0 and kt < KT - 2
                if nxt is not None:
                    ps_A, ps_B = scores_pair(nxt[0], nxt[1])
                    at_next = atp.tile([P, 2, QL], F16, tag="at")
                    emit_exp(nxt[0], nxt[1], ps_A, at_next, 0)
                    emit_exp(nxt[0], nxt[1], ps_B, at_next, 1)
                for fn, a in fills:
                    fn(*a)
                if m == 0 and kt == 0:
                    vt_proj(1)
                if has_vt:
                    vt_proj(kt + 2)
                if kt % 2 == 1:
                    # one wide add covers both halves: [P, 2, QL]
                    nc.vector.tensor_add(
                        s1[:, kt // 2, :, :], at_prev[:], at_t[:]
                    )
                else:
                    at_prev = at_t
                attnv(m, kt, po, at_t)
                if pending is not None and kt in (2, 3):
                    finish_tail(pending, kt)
                    if kt == 3:
                        pending = None
                if kt in (3, 7, 11) or (kt == 15 and m < DC - 1):
                    # wide L2 fold: s2[j] = s1[2j] + s1[2j+1], both halves
                    j = kt // 4
                    nc.vector.tensor_add(
                        s2[:, j, :, :], s1[:, 2 * j, :, :], s1[:, 2 * j + 1, :, :]
                    )

            otu = otup.tile([P, QL], F16, tag="otu", name=f"otu{m}")
            nc.vector.tensor_copy(otu[:], po[:])
            s3 = None
            if m < DC - 1:
                # L3+L4 folds cut the deferred rb matmuls to one source
                # (DVE epilogue slack); s3[1] is scratch for the L3 halves
                s3 = s3p.tile([P, 2, 2, QL], F16, tag="s3", name=f"s3_{m}")
                nc.vector.tensor_add(
                    s3[:, 0, :, :], s2[:, 0, :, :], s2[:, 1, :, :]
                )
                nc.vector.tensor_add(
                    s3[:, 1, :, :], s2[:, 2, :, :], s2[:, 3, :, :]
                )
                s4 = s4p.tile([P, 1, 2, QL], F16, tag="s4", name=f"s4_{m}")
                nc.vector.tensor_add(
                    s4[:, 0, :, :], s3[:, 0, :, :], s3[:, 1, :, :]
                )
                s3 = s4
            rbr = rbp.tile([P, QL], F32, tag="rbr", name=f"rbr{m}")
            pending = (m, otu, s2, s1, s3, rbr)

        def keep_warm(tag):
            # bridge PE-idle windows in the tail so HAM keeps the PE at full
            # clock for the output projection; results are never read.
            wp = pps.tile([P, N], F32, tag="ps", name=f"warm_{tag}")
            for i in range(3):
                nc.tensor.matmul(
                    wp[:], lhsT=warm_sb[:, 0:P], rhs=warm_sb[:],
                    start=(i == 0), stop=(i == 2),
                )

        # last m's tail runs immediately
        keep_warm("t0")
        finish_tail(pending, 2, last=True)
        keep_warm("t1")
        finish_tail(pending, 3, last=True)
        pending = None

        # ---- output projection + bias (qc-outer: qc0 overlaps qc1's
        # denominator chain) -------------------------------------------
        for qc in range(QC):
            for mo in range(DC):
                ps = pps.tile([P, QL], F32, tag="ps", name=f"op{mo}{qc}")
                for mh in range(DC):
                    nc.tensor.matmul(
                        ps[:, 0:N],
                        lhsT=wo_sb[:, mh, mo * P:(mo + 1) * P],
                        rhs=ot_sb[:, mh, qc * N:(qc + 1) * N],
                        start=(mh == 0), stop=(mh == DC - 1),
                    )
                ob = outp.tile([P, N], F16, tag="ob")
                if qc == 0:
                    nc.scalar.add(ob[:], ps[:, 0:N], bias_sb[:, mo:mo + 1])
                else:
                    nc.vector.tensor_scalar_add(
                        ob[:], ps[:, 0:N], bias_sb[:, mo:mo + 1]
                    )
                nc.sync.dma_start(
                    out=out[mo * P:(mo + 1) * P, qc * N:(qc + 1) * N], in_=ob[:]
                )


def build():
    nc = bacc.Bacc("TRN2", target_bir_lowering=False, debug=False)
    x = nc.dram_tensor("x", [D, L], F16, kind="ExternalInput").ap()
    wq = nc.dram_tensor("wq", [D, HID], F16, kind="ExternalInput").ap()
    wk = nc.dram_tensor("wk", [D, HID], F16, kind="ExternalInput").ap()
    wv = nc.dram_tensor("wv", [D, HID], F16, kind="ExternalInput").ap()
    wo = nc.dram_tensor("wo", [HID, D], F16, kind="ExternalInput").ap()
    bias = nc.dram_tensor("bias", [P, DC], F32, kind="ExternalInput").ap()
    out = nc.dram_tensor("out", [D, QL], F16, kind="ExternalOutput").ap()
    with tile.TileContext(nc) as tc:
        emit(nc, tc, x, wq, wk, wv, wo, bias, out)
    nc.compile()
    return nc


_NC_CACHE = None


def _get_nc():
    global _NC_CACHE
    if _NC_CACHE is None:
        _NC_CACHE = build()
    return _NC_CACHE


def make_in_maps(x, w_qkv, w_out, b_out):
    """Host-side sharding: returns the 8 per-core input dicts."""
    f16 = np.float16
    wq_t = np.ascontiguousarray((w_qkv[0:HID] * SCALE).T).astype(f16)
    wk_t = np.ascontiguousarray(w_qkv[HID:2 * HID].T).astype(f16)
    wv_t = np.ascontiguousarray(w_qkv[2 * HID:3 * HID].T).astype(f16)
    wo_t = np.ascontiguousarray(w_out.T).astype(f16)
    bias = np.ascontiguousarray(b_out.reshape(DC, P).T).astype(np.float32)
    in_maps = []
    for core in range(8):
        b, halfq = core // 2, core % 2
        # rotate so this core's query half sits at columns 0:QL; key order
        # is irrelevant (softmax sums over all keys).
        x_rot = np.roll(x[b], -halfq * QL, axis=1).astype(f16)
        in_maps.append({
            "x": np.ascontiguousarray(x_rot),
            "wq": wq_t, "wk": wk_t, "wv": wv_t, "wo": wo_t,
            "bias": bias,
        })
    return in_maps


def assemble(results):
    out = np.zeros((4, D, L), np.float32)
    for core in range(8):
        b, halfq = core // 2, core % 2
        out[b][:, halfq * QL:(halfq + 1) * QL] = results[core]["out"]
    return out


def kernel(x, w_qkv, w_out, b_out):
    x = np.asarray(x, np.float32)
    w_qkv = np.asarray(w_qkv, np.float32)
    w_out = np.asarray(w_out, np.float32)
    b_out = np.asarray(b_out, np.float32)
    nc = _get_nc()
    in_maps = make_in_maps(x, w_qkv, w_out, b_out)
    res = run_bass_kernel_spmd(nc, in_maps, core_ids=list(range(8)))
    return assemble(res.results)


# revision 81
# speedup vs baseline: 1.0236x; 1.0236x over previous
"""Trainium2 Bass kernel for multi-head attention (B=4, H=8, L=2048, dim=512).

Sharding: 8 cores = 4 batches x 2 query halves. Each core computes attention
for one batch's 1024-query half (all 8 heads) over the full 2048-key range;
no cross-core communication.

Design: the wall clock is the ScalarE exp stream (16.8M softmax elements at
1 elem/cycle/lane); everything else hides under it:
  - Q/K/V projections interleave into the attention kt-loop as PE filler;
    a short warm-up matmul burst releases the PE HAM clock-gate before the
    prologue projections run.
  - Scores: row-packed pairs (two 64-contraction matmuls in row groups 0-1 /
    2-3); attn@V: col-packed pairs accumulating PSUM over kt. PE order is
    S(kt+1) before AV(kt) with a 3-slot score-PSUM ring so the PE refills
    while exp(kt) drains.
  - exp tiles land in paired [128, 2, 1024] fp16 SBUF tiles so the L1
    denominator adds process both head-halves in one DVE op.
  - Denominator: L1 pairwise adds (8 per m) -> S1[8 slots]; the remaining
    reduction is an accumulating all-ones [128,64] stationary matmul over
    the 8 slots (PE), whose output rows all equal the key-sum, followed by
    fast reciprocal + normalize on DVE. Deferred into the next m's kt 2..3.
  - A tunable subset of (m,kt) A-halves uses a Schraudolph bit-trick exp
    (round(1477.32*s + 15300) as int16 == fp16 bits): DVE casts PSUM->fp16,
    GpSimd does the fused mul+add+round. ~4% max elementwise error on those
    tiles, <1e-2 on the final output.
"""
import numpy as np

import concourse.bass as bass
import concourse.tile as tile
from concourse import bacc, mybir
from concourse.bass_utils import run_bass_kernel_spmd

F16 = mybir.dt.float16
F32 = mybir.dt.float32
I16 = mybir.dt.int16
EXP = mybir.ActivationFunctionType.Exp
CPY = mybir.ActivationFunctionType.Copy
MUL = mybir.AluOpType.mult
ADD = mybir.AluOpType.add

P = 128
D = 512          # model dim
L = 2048         # keys
QL = 1024        # per-core queries
H = 8
C = 64           # head dim
HID = 512
DC = D // P      # 4 contraction chunks
KT = L // P      # 16 key tiles
N = 512          # matmul free-dim chunk
QC = QL // N     # 2 query chunks
LC = L // N      # 4 key chunks
SCALE = C ** -0.5

A_SCH = 1477.319722        # 1024/ln(2)
B_SCH = 15300.0            # fp16 exp bias 15360 - sigma* (60)

# kt's whose A-half exp uses the approximate bit-trick path, computed
# entirely on DVE straight from PSUM; B-half stays exact on ScalarE.
# m0 stays exact: it is PE/DVE-bound (V-projection casts live there) and
# its ScalarE has slack, so approx would cost DVE time for nothing.
APPROX_KT = {
    0: frozenset(),
    1: frozenset({4, 5, 6, 8, 10, 11, 14}),
    2: frozenset({4, 5, 6, 8, 10, 11, 14}),
    3: frozenset({4, 5, 6, 8, 10, 11, 14}),
}


def emit(nc, tc, x, wq, wk, wv, wo, bias, out):
    import contextlib
    ctx = contextlib.ExitStack()
    with ctx:
        # ---- pools -----------------------------------------------------
        consts = ctx.enter_context(tc.tile_pool(name="consts", bufs=1))
        qkv = ctx.enter_context(tc.tile_pool(name="qkv", bufs=1))
        ph1 = ctx.enter_context(tc.tile_pool(name="ph1", bufs=1))
        atp = ctx.enter_context(tc.tile_pool(name="atp", bufs=4))
        stp = ctx.enter_context(tc.tile_pool(name="stp", bufs=2))
        s1p = ctx.enter_context(tc.tile_pool(name="s1p", bufs=1))
        s2p = ctx.enter_context(tc.tile_pool(name="s2p", bufs=2))
        s3p = ctx.enter_context(tc.tile_pool(name="s3p", bufs=2))
        rbp = ctx.enter_context(tc.tile_pool(name="rbp", bufs=2))
        otup = ctx.enter_context(tc.tile_pool(name="otup", bufs=2))
        outp = ctx.enter_context(tc.tile_pool(name="outp", bufs=2))
        # PSUM: shared 3-slot ring (6 banks) for scores + projection/rb/
        # out-proj tiles, + po 2 banks = 8 banks.
        pps = ctx.enter_context(tc.tile_pool(name="pps", bufs=3, space="PSUM"))
        ppo = ctx.enter_context(tc.tile_pool(name="ppo", bufs=1, space="PSUM"))

        # ---- persistent SBUF ------------------------------------------
        wo_sb = consts.tile([P, DC, HID], F16)
        bias_sb = consts.tile([P, DC], F32)
        ones_sb = consts.tile([P, C], F16)
        warm_sb = consts.tile([P, N], F16)
        nc.vector.memset(ones_sb[:], 1.0)
        nc.vector.memset(warm_sb[:], 0.25)
        # load the exp table while DMAs stream
        nc.scalar.activation(warm_sb[:, 0:1], warm_sb[:, 0:1], EXP)

        q_sb = qkv.tile([P, DC, QL], F16)
        k_sb = qkv.tile([P, DC, L], F16)
        vt_sb = qkv.tile([P, KT, HID], F16)
        ot_sb = qkv.tile([P, DC, QL], F16)

        x_sb = ph1.tile([P, DC, L], F16)
        wq_sb = ph1.tile([P, DC, HID], F16)
        wk_sb = ph1.tile([P, DC, HID], F16)
        wv_sb = ph1.tile([P, DC, HID], F16)
        xr = x.rearrange("(a p) n -> p a n", p=P)
        nc.sync.dma_start(out=wq_sb[:], in_=wq.rearrange("(a p) n -> p a n", p=P))
        nc.sync.dma_start(out=x_sb[:, :, 0:N], in_=xr[:, :, 0:N])
        nc.sync.dma_start(out=x_sb[:, :, N:QL], in_=xr[:, :, N:QL])
        nc.sync.dma_start(out=wk_sb[:], in_=wk.rearrange("(a p) n -> p a n", p=P))
        nc.sync.dma_start(out=wv_sb[:], in_=wv.rearrange("(a p) n -> p a n", p=P))
        nc.sync.dma_start(out=x_sb[:, :, QL:QL + N], in_=xr[:, :, QL:QL + N])
        nc.sync.dma_start(out=x_sb[:, :, QL + N:L], in_=xr[:, :, QL + N:L])
        nc.sync.dma_start(out=wo_sb[:], in_=wo.rearrange("(a p) n -> p a n", p=P))
        nc.sync.dma_start(out=bias_sb[:], in_=bias)

        # HAM warm-up: ~9 dummy matmuls (~3.5us cold) release the PE clock
        # gate so the prologue projections run at 2.4 GHz. Results unused.
        warm_ps = pps.tile([P, N], F32, tag="ps", name="warm_ps")
        for i in range(9):
            nc.tensor.matmul(
                warm_ps[:], lhsT=warm_sb[:, 0:P], rhs=warm_sb[:],
                start=(i == 0), stop=(i == 8),
            )

        # ---- projection groups (PE filler) -----------------------------
        def q_proj(m, qc):
            ps = pps.tile([P, N], F32, tag="ps", name=f"q{m}{qc}")
            for dc in range(DC):
                nc.tensor.matmul(
                    ps[:], lhsT=wq_sb[:, dc, m * P:(m + 1) * P],
                    rhs=x_sb[:, dc, qc * N:(qc + 1) * N],
                    start=(dc == 0), stop=(dc == DC - 1),
                )
            nc.vector.tensor_copy(q_sb[:, m, qc * N:(qc + 1) * N], ps[:])

        def k_proj(m, lc):
            ps = pps.tile([P, N], F32, tag="ps", name=f"k{m}{lc}")
            for dc in range(DC):
                nc.tensor.matmul(
                    ps[:], lhsT=wk_sb[:, dc, m * P:(m + 1) * P],
                    rhs=x_sb[:, dc, lc * N:(lc + 1) * N],
                    start=(dc == 0), stop=(dc == DC - 1),
                )
            nc.vector.tensor_copy(k_sb[:, m, lc * N:(lc + 1) * N], ps[:])

        def vt_proj(kt):
            ps = pps.tile([P, N], F32, tag="ps", name=f"vt{kt}")
            for dc in range(DC):
                nc.tensor.matmul(
                    ps[:], lhsT=x_sb[:, dc, kt * P:(kt + 1) * P],
                    rhs=wv_sb[:, dc, :],
                    start=(dc == 0), stop=(dc == DC - 1),
                )
            # DVE-only: m0's ScalarE is saturated by the exp stream; casts
            # there would make it the m0 bottleneck
            nc.vector.tensor_copy(vt_sb[:, kt, :], ps[:])

        fill = {}

        def add_fill(m, kt, fn, *a):
            fill.setdefault((m, kt), []).append((fn, a))

        # fillers sit on approx kts where ScalarE idles and score PSUM
        # frees fast; K(m,lc) must land before kt=4*lc.
        add_fill(0, 1, k_proj, 0, 1)
        add_fill(0, 5, k_proj, 0, 2)
        add_fill(0, 8, k_proj, 0, 3)
        add_fill(0, 11, q_proj, 1, 0)
        add_fill(0, 13, q_proj, 1, 1)
        add_fill(0, 14, k_proj, 1, 0)
        for m in (1, 2):
            add_fill(m, 1, k_proj, m, 1)
            add_fill(m, 5, k_proj, m, 2)
            add_fill(m, 8, k_proj, m, 3)
            add_fill(m, 11, q_proj, m + 1, 0)
            add_fill(m, 14, q_proj, m + 1, 1)
            add_fill(m, 6, k_proj, m + 1, 0)
        add_fill(3, 1, k_proj, 3, 1)
        add_fill(3, 5, k_proj, 3, 2)
        add_fill(3, 8, k_proj, 3, 3)

        # ---- exp dispatch ----------------------------------------------
        def emit_exp(m, kt, ps_t, at_t, half):
            dst = at_t[:, half, :]
            if half == 0 and kt in APPROX_KT[m]:
                nc.vector.tensor_scalar(
                    dst.bitcast(I16), ps_t[:], A_SCH, B_SCH, MUL, ADD,
                )
            else:
                nc.scalar.activation(dst, ps_t[:], EXP)

        def scores_half(m, kt, half):
            # one row-packed half (A: rows 0:64 at (0,0), B: 64:128 at (64,0))
            lo = half * C
            ps_t = pps.tile([P, QL], F32, tag="ps", name=f"ps{half}")
            for qc in range(QC):
                nc.tensor.matmul(
                    ps_t[:, qc * N:(qc + 1) * N],
                    lhsT=k_sb[lo:lo + C, m, kt * P:(kt + 1) * P],
                    rhs=q_sb[lo:lo + C, m, qc * N:(qc + 1) * N],
                    start=True, stop=True, tile_position=(lo, 0),
                )
            return ps_t

        def scores_pair(m, kt):
            # interleave A/B so the row-packed pairs stream concurrently
            # (used on kts whose ring slots are double-buffered)
            ps_A = pps.tile([P, QL], F32, tag="ps")
            ps_B = pps.tile([P, QL], F32, tag="ps")
            for qc in range(QC):
                nc.tensor.matmul(
                    ps_A[:, qc * N:(qc + 1) * N],
                    lhsT=k_sb[0:C, m, kt * P:(kt + 1) * P],
                    rhs=q_sb[0:C, m, qc * N:(qc + 1) * N],
                    start=True, stop=True, tile_position=(0, 0),
                )
                nc.tensor.matmul(
                    ps_B[:, qc * N:(qc + 1) * N],
                    lhsT=k_sb[C:P, m, kt * P:(kt + 1) * P],
                    rhs=q_sb[C:P, m, qc * N:(qc + 1) * N],
                    start=True, stop=True, tile_position=(C, 0),
                )
            return ps_A, ps_B

        def attnv(m, kt, po, at_t):
            for qc in range(QC):
                nc.tensor.matmul(
                    po[0:C, qc * N:(qc + 1) * N],
                    lhsT=vt_sb[:, kt, (2 * m) * C:(2 * m + 1) * C],
                    rhs=at_t[:, 0, qc * N:(qc + 1) * N],
                    start=(kt == 0), stop=(kt == KT - 1),
                    tile_position=(0, 0), skip_group_check=True,
                )
                nc.tensor.matmul(
                    po[C:P, qc * N:(qc + 1) * N],
                    lhsT=vt_sb[:, kt, (2 * m + 1) * C:(2 * m + 2) * C],
                    rhs=at_t[:, 1, qc * N:(qc + 1) * N],
                    start=(kt == 0), stop=(kt == KT - 1),
                    tile_position=(0, C), skip_group_check=True,
                )

        def finish_tail(st, kt_slot, last=False):
            # deferred denominators: accumulate the fold outputs through the
            # all-ones stationary matmul (rows land pre-broadcast), then
            # fast reciprocal + normalize. The non-deferred last-m tail
            # reads s2/s1 directly so the final folds stay off its chain.
            m, otu, s2, s1, s3, rbr = st
            qc = kt_slot - 2
            if last:
                srcs = [(s2, 0), (s2, 1), (s2, 2), (s1, 6), (s1, 7)]
            else:
                srcs = [(s3, 0), (s3, 1)]
            rb_ps = pps.tile([P, N], F32, tag="ps", name=f"rb{m}_{qc}")
            for step, (src, j) in enumerate(srcs):
                for half in range(2):
                    nc.tensor.matmul(
                        rb_ps[half * C:(half + 1) * C, :],
                        lhsT=ones_sb[:],
                        rhs=src[:, j, half, qc * N:(qc + 1) * N],
                        start=(step == 0), stop=(step == len(srcs) - 1),
                        tile_position=(0, half * C), skip_group_check=True,
                    )
            nc.vector.reciprocal_approx_fast(
                out=rbr[:, qc * N:(qc + 1) * N], in_=rb_ps[:]
            )
            nc.vector.tensor_mul(
                ot_sb[:, m, qc * N:(qc + 1) * N], otu[:, qc * N:(qc + 1) * N],
                rbr[:, qc * N:(qc + 1) * N],
            )

        # ---- prologue (vt(1) rides inside the kt loop, under the first
        # exps, instead of extending the exp-idle prologue) ----------------
        q_proj(0, 0)
        q_proj(0, 1)
        k_proj(0, 0)
        vt_proj(0)

        # ---- main loop --------------------------------------------------
        pending = None
        at_next = None
        for m in range(DC):
            po = ppo.tile([P, QL], F32, tag="po", name=f"po{m}")
            s1 = s1p.tile([P, KT // 2, 2, QL], F16, tag="s1", name=f"s1_{m}")
            s2 = s2p.tile([P, KT // 4, 2, QL], F16, tag="s2", name=f"s2_{m}")
            at_prev = None
            if at_next is None:
                ps_A0, ps_B0 = scores_pair(m, 0)
                at_next = atp.tile([P, 2, QL], F16, tag="at")
                emit_exp(m, 0, ps_A0, at_next, 0)
                emit_exp(m, 0, ps_B0, at_next, 1)
            for kt in range(KT):
                at_t = at_next
                # refill score PSUM for kt+1 (or next m's kt 0) before AV(kt).
                # On filler kts, fillers are emitted BETWEEN the A-half and
                # B-half of the next scores: the PE FIFO stalls there anyway
                # (B waits exp_B of the previous kt), so the filler matmuls
                # execute inside that window instead of queueing behind it.
                # On clean kts the interleaved pair keeps row-pack concurrency.
                nxt = None
                if kt < KT - 1:
                    nxt = (m, kt + 1)
                elif m < DC - 1:
                    nxt = (m + 1, 0)
                fills = list(fill.get((m, kt), ()))
                has_vt = m == 0 and kt < KT - 2
                if nxt is not None:
                    ps_A, ps_B = scores_pair(nxt[0], nxt[1])
                    at_next = atp.tile([P, 2, QL], F16, tag="at")
                    emit_exp(nxt[0], nxt[1], ps_A, at_next, 0)
                    emit_exp(nxt[0], nxt[1], ps_B, at_next, 1)
                for fn, a in fills:
                    fn(*a)
                if m == 0 and kt == 0:
                    vt_proj(1)
                if has_vt:
                    vt_proj(kt + 2)
                if kt % 2 == 1:
                    # one wide add covers both halves: [P, 2, QL]
                    nc.vector.tensor_add(
                        s1[:, kt // 2, :, :], at_prev[:], at_t[:]
                    )
                else:
                    at_prev = at_t
                attnv(m, kt, po, at_t)
                if pending is not None and kt in (2, 3):
                    finish_tail(pending, kt)
                    if kt == 3:
                        pending = None
                if kt in (3, 7, 11) or (kt == 15 and m < DC - 1):
                    # wide L2 fold: s2[j] = s1[2j] + s1[2j+1], both halves
                    j = kt // 4
                    nc.vector.tensor_add(
                        s2[:, j, :, :], s1[:, 2 * j, :, :], s1[:, 2 * j + 1, :, :]
                    )

            otu = otup.tile([P, QL], F16, tag="otu", name=f"otu{m}")
            nc.vector.tensor_copy(otu[:], po[:])
            s3 = None
            if m < DC - 1:
                # L3 folds halve the deferred rb matmul count (DVE has slack)
                s3 = s3p.tile([P, 2, 2, QL], F16, tag="s3", name=f"s3_{m}")
                nc.vector.tensor_add(
                    s3[:, 0, :, :], s2[:, 0, :, :], s2[:, 1, :, :]
                )
                nc.vector.tensor_add(
                    s3[:, 1, :, :], s2[:, 2, :, :], s2[:, 3, :, :]
                )
            rbr = rbp.tile([P, QL], F32, tag="rbr", name=f"rbr{m}")
            pending = (m, otu, s2, s1, s3, rbr)

        def keep_warm(tag):
            # bridge PE-idle windows in the tail so HAM keeps the PE at full
            # clock for the output projection; results are never read.
            wp = pps.tile([P, N], F32, tag="ps", name=f"warm_{tag}")
            for i in range(3):
                nc.tensor.matmul(
                    wp[:], lhsT=warm_sb[:, 0:P], rhs=warm_sb[:],
                    start=(i == 0), stop=(i == 2),
                )

        # last m's tail runs immediately
        keep_warm("t0")
        finish_tail(pending, 2, last=True)
        keep_warm("t1")
        finish_tail(pending, 3, last=True)
        pending = None

        # ---- output projection + bias (qc-outer: qc0 overlaps qc1's
        # denominator chain) -------------------------------------------
        for qc in range(QC):
            for mo in range(DC):
                ps = pps.tile([P, QL], F32, tag="ps", name=f"op{mo}{qc}")
                for mh in range(DC):
                    nc.tensor.matmul(
                        ps[:, 0:N],
                        lhsT=wo_sb[:, mh, mo * P:(mo + 1) * P],
                        rhs=ot_sb[:, mh, qc * N:(qc + 1) * N],
                        start=(mh == 0), stop=(mh == DC - 1),
                    )
                ob = outp.tile([P, N], F16, tag="ob")
                if qc == 0:
                    nc.scalar.add(ob[:], ps[:, 0:N], bias_sb[:, mo:mo + 1])
                else:
                    nc.vector.tensor_scalar_add(
                        ob[:], ps[:, 0:N], bias_sb[:, mo:mo + 1]
                    )
                nc.sync.dma_start(
                    out=out[mo * P:(mo + 1) * P, qc * N:(qc + 1) * N], in_=ob[:]
                )


def build():
    nc = bacc.Bacc("TRN2", target_bir_lowering=False, debug=False)
    x = nc.dram_tensor("x", [D, L], F16, kind="ExternalInput").ap()
    wq = nc.dram_tensor("wq", [D, HID], F16, kind="ExternalInput").ap()
    wk = nc.dram_tensor("wk", [D, HID], F16, kind="ExternalInput").ap()
    wv = nc.dram_tensor("wv", [D, HID], F16, kind="ExternalInput").ap()
    wo = nc.dram_tensor("wo", [HID, D], F16, kind="ExternalInput").ap()
    bias = nc.dram_tensor("bias", [P, DC], F32, kind="ExternalInput").ap()
    out = nc.dram_tensor("out", [D, QL], F16, kind="ExternalOutput").ap()
    with tile.TileContext(nc) as tc:
        emit(nc, tc, x, wq, wk, wv, wo, bias, out)
    nc.compile()
    return nc


_NC_CACHE = None


def _get_nc():
    global _NC_CACHE
    if _NC_CACHE is None:
        _NC_CACHE = build()
    return _NC_CACHE


def make_in_maps(x, w_qkv, w_out, b_out):
    """Host-side sharding: returns the 8 per-core input dicts."""
    f16 = np.float16
    wq_t = np.ascontiguousarray((w_qkv[0:HID] * SCALE).T).astype(f16)
    wk_t = np.ascontiguousarray(w_qkv[HID:2 * HID].T).astype(f16)
    wv_t = np.ascontiguousarray(w_qkv[2 * HID:3 * HID].T).astype(f16)
    wo_t = np.ascontiguousarray(w_out.T).astype(f16)
    bias = np.ascontiguousarray(b_out.reshape(DC, P).T).astype(np.float32)
    in_maps = []
    for core in range(8):
        b, halfq = core // 2, core % 2
        # rotate so this core's query half sits at columns 0:QL; key order
        # is irrelevant (softmax sums over all keys).
        x_rot = np.roll(x[b], -halfq * QL, axis=1).astype(f16)
        in_maps.append({
            "x": np.ascontiguousarray(x_rot),
            "wq": wq_t, "wk": wk_t, "wv": wv_t, "wo": wo_t,
            "bias": bias,
        })
    return in_maps


def assemble(results):
    out = np.zeros((4, D, L), np.float32)
    for core in range(8):
        b, halfq = core // 2, core % 2
        out[b][:, halfq * QL:(halfq + 1) * QL] = results[core]["out"]
    return out


def kernel(x, w_qkv, w_out, b_out):
    x = np.asarray(x, np.float32)
    w_qkv = np.asarray(w_qkv, np.float32)
    w_out = np.asarray(w_out, np.float32)
    b_out = np.asarray(b_out, np.float32)
    nc = _get_nc()
    in_maps = make_in_maps(x, w_qkv, w_out, b_out)
    res = run_bass_kernel_spmd(nc, in_maps, core_ids=list(range(8)))
    return assemble(res.results)
